# revision 1
# baseline (speedup 1.0000x reference)
"""Trainium2 Bass kernel for nn_MixerBlock (gnn_message_passing).

Sharding: 8 cores = (batch b in 0..3) x (channel-half h in 0..1).
Each core runs the Chebyshev chains for its (b, h) shard [N, 128]: the
8-neighbor gather uses SWDGE dma_gather over bf16 rows in HBM, and the
weighted k-reduction runs on the TensorEngine via block-diagonal-weight
matmuls accumulated in PSUM. Cross-pair traffic (conv partial sums, LN
stats, z4 halves) uses AllToAll / AllReduce collectives on pair groups.

The SPMD program is identical on all cores; all (b, h) differences are
carried by host-staged input data (column/row permutations so that each
core's own channel half always comes first).
"""
import numpy as np
import ml_dtypes

bf16 = ml_dtypes.bfloat16

B, N, C, K, NB, T, CHID = 4, 12288, 256, 6, 8, 512, 1024
CH = C // 2            # 128, per-core channel half
NT = N // 128          # 96 node tiles
EPS = 1e-6
NCORES = 8
GCHUNK_TILES = 8
GCHUNK_IDXS = GCHUNK_TILES * 128 * NB   # 8192
NCHUNKS = NT // GCHUNK_TILES            # 12

_cache = {}


def _u16(x):
    return np.ascontiguousarray(x).view(np.uint16)


def host_prep(inputs):
    maps = np.asarray(inputs["maps"], np.float32)
    idx = np.asarray(inputs["neigh_idx"], np.int32)
    w = np.asarray(inputs["neigh_w"], np.float32)
    diag = np.asarray(inputs["diag_w"], np.float32)

    tt, gg, hh, ii, kk = np.meshgrid(
        np.arange(NT), np.arange(4), np.arange(2), np.arange(32), np.arange(4),
        indexing="ij")
    nidx = (tt * 128 + gg * 32 + ii).ravel()
    kidx = (hh * 4 + kk).ravel()
    idxlist = idx[nidx, kidx].astype(np.int16)
    NUM = N * NB
    idxwrap = np.zeros((128, NUM // 16), np.int16)
    base = idxlist.reshape(NUM // 16, 16).T
    for g8 in range(8):
        idxwrap[16 * g8:16 * g8 + 16, :] = base

    Wblk = np.zeros((128, NT * 256), np.float32)
    cols = (tt * 256 + (gg * 2 + hh) * 32 + ii).ravel()
    rows = (ii * 4 + kk).ravel()
    Wblk[rows, cols] = w[nidx, kidx]
    Wblk = Wblk.astype(bf16)

    diag_b = diag.astype(bf16).astype(np.float32)
    diag1 = np.ascontiguousarray(diag_b.reshape(NT, 128).T)
    diag2 = np.ascontiguousarray((2.0 * diag_b).reshape(NT, 128).T)

    negI = (-0.5 * np.eye(128)).astype(bf16)
    zI = np.zeros((128, 128), bf16)
    identb = np.eye(128).astype(bf16)

    conv1_w = np.asarray(inputs["conv1_w"], np.float32)
    conv2_w = np.asarray(inputs["conv2_w"], np.float32)
    tok_w1 = np.asarray(inputs["tok_w1"], np.float32).astype(bf16)
    tok_w2 = np.asarray(inputs["tok_w2"], np.float32).astype(bf16)
    ch_w1 = np.asarray(inputs["ch_w1"], np.float32).astype(bf16)
    ch_w2 = np.asarray(inputs["ch_w2"], np.float32)

    b1bc = np.ascontiguousarray(np.broadcast_to(
        np.asarray(inputs["tok_b1"], np.float32), (128, T)))
    tokb2 = np.ascontiguousarray(
        np.asarray(inputs["tok_b2"], np.float32).reshape(NT, 128).T)
    chb1 = np.ascontiguousarray(
        np.asarray(inputs["ch_b1"], np.float32).reshape(8, 128).T)

    ln_trivial = {}
    for nm in ("ln1", "ln2", "ln3", "ln4"):
        sc = np.asarray(inputs[nm + "_scale"], np.float32)
        bi = np.asarray(inputs[nm + "_bias"], np.float32)
        ln_trivial[nm] = bool(np.allclose(sc, 1.0) and np.allclose(bi, 0.0))

    in_maps = []
    for cid in range(NCORES):
        b, h = cid // 2, cid % 2
        own = slice(h * CH, (h + 1) * CH)
        oth = slice((1 - h) * CH, (2 - h) * CH)
        perm = np.r_[np.arange(h * CH, (h + 1) * CH),
                     np.arange((1 - h) * CH, (2 - h) * CH)]
        W1h = np.stack([conv1_w[j * C + h * CH: j * C + (h + 1) * CH, :][:, perm]
                        for j in range(K)]).astype(bf16)
        W2h = np.stack([conv2_w[j * C + h * CH: j * C + (h + 1) * CH, :][:, perm]
                        for j in range(K)]).astype(bf16)
        convb = np.stack([
            np.broadcast_to(np.asarray(inputs["conv1_b"], np.float32)[own], (128, CH)),
            np.broadcast_to(np.asarray(inputs["conv2_b"], np.float32)[own], (128, CH)),
        ])
        lnsc = np.stack([np.broadcast_to(
            np.asarray(inputs[nm + "_scale"], np.float32)[perm], (128, C))
            for nm in ("ln1", "ln2", "ln3", "ln4")])
        lnbi = np.stack([np.broadcast_to(
            np.asarray(inputs[nm + "_bias"], np.float32)[perm], (128, C))
            for nm in ("ln1", "ln2", "ln3", "ln4")])
        m = {
            "maps_nat": np.ascontiguousarray(maps[b][:, perm]),
            "idxwrap": idxwrap,
            "wblk": _u16(Wblk),
            "diag1": diag1, "diag2": diag2,
            "negI": _u16(negI), "zI": _u16(zI), "identb": _u16(identb),
            "w1h": _u16(W1h), "w2h": _u16(W2h),
            "convb": np.ascontiguousarray(convb),
            "tokw1": _u16(tok_w1), "tokw2": _u16(tok_w2),
            "b1bc": b1bc, "tokb2": tokb2,
            "chw1": _u16(ch_w1), "chb1": chb1,
            "chw2h": _u16(np.ascontiguousarray(ch_w2[:, own]).astype(bf16)),
            "chb2bc": np.ascontiguousarray(np.broadcast_to(
                np.asarray(inputs["ch_b2"], np.float32)[own], (128, CH))),
            "lnsc": np.ascontiguousarray(lnsc),
            "lnbi": np.ascontiguousarray(lnbi),
        }
        in_maps.append(m)
    return in_maps, ln_trivial


def build_nc(num_devices, ln_trivial, native_gelu=True, stop_stage=99):
    import concourse.bass as bass
    import concourse.bacc as bacc
    import concourse.mybir as mybir
    import concourse.tile as tile
    from contextlib import ExitStack

    dt = mybir.dt
    AF = mybir.ActivationFunctionType
    OP = mybir.AluOpType

    nc = bacc.Bacc("TRN2", target_bir_lowering=False, debug=False,
                   num_devices=num_devices)

    def din(name, shape, dtype):
        return nc.dram_tensor(name, shape, dtype, kind="ExternalInput")

    maps_nat = din("maps_nat", [N, C], dt.float32)
    idxwrap = din("idxwrap", [128, N * NB // 16], dt.int16)
    wblk_d = din("wblk", [128, NT * 256], dt.uint16)
    diag1_d = din("diag1", [128, NT], dt.float32)
    diag2_d = din("diag2", [128, NT], dt.float32)
    negI_d = din("negI", [128, 128], dt.uint16)
    zI_d = din("zI", [128, 128], dt.uint16)
    identb_d = din("identb", [128, 128], dt.uint16)
    w1h_d = din("w1h", [K, CH, C], dt.uint16)
    w2h_d = din("w2h", [K, CH, C], dt.uint16)
    convb_d = din("convb", [2, 128, CH], dt.float32)
    tokw1_d = din("tokw1", [N, T], dt.uint16)
    tokw2_d = din("tokw2", [T, N], dt.uint16)
    b1bc_d = din("b1bc", [128, T], dt.float32)
    tokb2_d = din("tokb2", [128, NT], dt.float32)
    chw1_d = din("chw1", [C, CHID], dt.uint16)
    chb1_d = din("chb1", [128, 8], dt.float32)
    chw2h_d = din("chw2h", [CHID, CH], dt.uint16)
    chb2bc_d = din("chb2bc", [128, CH], dt.float32)
    lnsc_d = din("lnsc", [4, 128, C], dt.float32)
    lnbi_d = din("lnbi", [4, 128, C], dt.float32)

    outp = nc.dram_tensor("outp", [N, CH], dt.float32, kind="ExternalOutput")

    RG = [[0, 1], [2, 3], [4, 5], [6, 7]]

    with tile.TileContext(nc) as tc:
        es = ExitStack()
        dram = es.enter_context(tc.tile_pool(name="dram", bufs=1, space="DRAM"))
        const = es.enter_context(tc.tile_pool(name="const", bufs=1))

        rows = [dram.tile([N, CH], dt.bfloat16, tag=f"rows{i}", name=f"rows{i}") for i in range(2)]
        tT = [dram.tile([CH, N], dt.bfloat16, tag=f"tT{j}", name=f"tT{j}") for j in range(K)]
        x1_dram = dram.tile([N, CH], dt.float32, tag="x1d", name="x1d")
        give1 = dram.tile([N, CH], dt.bfloat16, tag="give1", name="give1")
        a2a1 = dram.tile([2, N, CH], dt.bfloat16, tag="a2a1", name="a2a1")
        give2 = dram.tile([N, CH], dt.bfloat16, tag="give2", name="give2")
        a2a2 = dram.tile([2, N, CH], dt.bfloat16, tag="a2a2", name="a2a2")
        z4give = dram.tile([N, CH], dt.bfloat16, tag="z4give", name="z4give")
        a2az4 = dram.tile([2, N, CH], dt.bfloat16, tag="a2az4", name="a2az4")
        stat_in = [dram.tile([128, 2 * NT], dt.float32, tag=f"sti{i}", name=f"sti{i}") for i in range(3)]
        stat_out = [dram.tile([128, 2 * NT], dt.float32, tag=f"sto{i}", name=f"sto{i}") for i in range(3)]

        rows_ap = [r[:] for r in rows]
        tT_ap = [t[:] for t in tT]

        # ---- persistent SBUF constants ----
        wblk = const.tile([128, NT * 256], dt.bfloat16)
        nc.sync.dma_start(wblk[:], wblk_d.ap().bitcast(dt.bfloat16))
        idxt = const.tile([128, N * NB // 16], dt.int16)
        nc.sync.dma_start(idxt[:], idxwrap.ap())
        d1 = const.tile([128, NT], dt.float32)
        nc.sync.dma_start(d1[:], diag1_d.ap())
        d2 = const.tile([128, NT], dt.float32)
        nc.sync.dma_start(d2[:], diag2_d.ap())
        nI = const.tile([128, 128], dt.bfloat16)
        nc.sync.dma_start(nI[:], negI_d.ap().bitcast(dt.bfloat16))
        zIs = const.tile([128, 128], dt.bfloat16)
        nc.sync.dma_start(zIs[:], zI_d.ap().bitcast(dt.bfloat16))
        idn = const.tile([128, 128], dt.bfloat16)
        nc.sync.dma_start(idn[:], identb_d.ap().bitcast(dt.bfloat16))
        w1h = const.tile([128, K, C], dt.bfloat16)
        nc.sync.dma_start(w1h[:], w1h_d.ap().bitcast(dt.bfloat16).rearrange("k p c -> p k c"))
        w2h = const.tile([128, K, C], dt.bfloat16)
        nc.sync.dma_start(w2h[:], w2h_d.ap().bitcast(dt.bfloat16).rearrange("k p c -> p k c"))
        eps_col = const.tile([128, 1], dt.float32)
        nc.vector.memset(eps_col[:], EPS)

        maps_t = maps_nat.ap().rearrange("(t p) c -> p t c", p=128)
        x1_t = x1_dram[:].rearrange("(t p) c -> p t c", p=128)

        def transpose_to_tT(src_ap, tslot_ap, t, sb, psp):
            pt = psp.tile([128, 128], dt.bfloat16, tag="ptr", name="ptr")
            nc.tensor.transpose(pt[:], src_ap, idn[:])
            ot = sb.tile([128, 128], dt.bfloat16, tag="otr", name="otr")
            nc.scalar.activation(ot[:], pt[:], AF.Copy)
            nc.sync.dma_start(tslot_ap[:, t * 128:(t + 1) * 128], ot[:])

        def gelu_act(out_ap, in_ap, sb, bias=0.0):
            if native_gelu:
                nc.scalar.activation(out_ap, in_ap, AF.Gelu_apprx_tanh, bias=bias)
            else:
                sgt = sb.tile(list(in_ap.shape), dt.float32, tag="gelu_sg", name="gelu_sg")
                nc.scalar.activation(sgt[:], in_ap, AF.Sigmoid, scale=1.702, bias=bias)
                nc.vector.tensor_tensor(out_ap, in_ap, sgt[:], OP.mult)

        def run_chain(ring, sb, gbp, psp):
            """ring[0] = t0 (filled); rows[0] = t0 rows; tT[0] written."""
            for j in range(1, K):
                src = rows_ap[(j + 1) % 2]
                dst = rows_ap[j % 2]
                tcur = ring[(j - 1) % 3]
                tprev = ring[(j - 2) % 3] if j >= 2 else None
                tnext = ring[j % 3]
                dcol = d2 if j >= 2 else d1
                scale = 2.0 if j >= 2 else 1.0
                for chk in range(NT):
                    gb = gbp.tile([128, 8, CH], dt.bfloat16, tag="gb", name="gb")
                    nc.gpsimd.dma_gather(
                        out_ap=gb[:],
                        in_ap=src,
                        idxs_ap=idxt[:, chk * 64:(chk + 1) * 64],
                        num_idxs=1024,
                        num_idxs_reg=1024,
                        elem_size=CH,
                        single_packet=True,
                    )
                    for tl in range(1):
                        t = chk
                        ps = psp.tile([128, CH], dt.float32, tag="cps", name="cps")
                        if j >= 2:
                            nc.tensor.matmul(ps[:], nI[:], tprev[:, t, :], start=True, stop=True)
                        else:
                            nc.tensor.matmul(ps[:], zIs[:], tcur[:, t, :], start=True, stop=True)
                        for grp in range(4):
                            for kh in range(2):
                                wcol = t * 256 + (grp * 2 + kh) * 32
                                nc.tensor.matmul(
                                    ps[32 * grp:32 * grp + 32, :],
                                    wblk[:, wcol:wcol + 32],
                                    gb[:, grp * 2 + kh, :],
                                    start=False, stop=False, skip_group_check=True,
                                    tile_position=(0, 32 * grp),
                                )
                        u = sb.tile([128, CH], dt.float32, tag="cu", name="cu")
                        nc.scalar.activation(u[:], ps[:], AF.Copy, scale=scale)
                        nc.vector.scalar_tensor_tensor(
                            out=tnext[:, t, :], in0=tcur[:, t, :],
                            scalar=dcol[:, t:t + 1], in1=u[:],
                            op0=OP.mult, op1=OP.add)
                        nc.sync.dma_start(
                            dst.rearrange("(t p) c -> p t c", p=128)[:, t, :],
                            tnext[:, t, :])
                        transpose_to_tT(tnext[:, t, :], tT_ap[j], t, sb, psp)

        def conv_phase(whalf, give_ap, ymine, giveS, sb, psp):
            for t in range(NT):
                ps = psp.tile([128, C], dt.float32, tag="vps", name="vps")
                for j in range(K):
                    lt = sb.tile([128, 128], dt.bfloat16, tag="vlt", name="vlt")
                    nc.sync.dma_start(lt[:], tT_ap[j][:, t * 128:(t + 1) * 128])
                    nc.tensor.matmul(ps[:], lt[:], whalf[:, j, :],
                                     start=(j == 0), stop=(j == K - 1))
                nc.scalar.activation(ymine[:, t, :], ps[:, 0:CH], AF.Copy)
                nc.scalar.activation(giveS[:, t, :], ps[:, CH:C], AF.Copy)
                nc.sync.dma_start(
                    give_ap.rearrange("(t p) c -> p t c", p=128)[:, t, :],
                    giveS[:, t, :])

        def postconv(a2a_ap, conv_i, stat_i, zout, ymine, giveS, big, sb):
            """y = ymine + ag[0] + ag[1] - give + bias -> gelu -> pair LN -> zout."""
            sall = big.tile([128, 2 * NT], dt.float32, tag=f"psall{stat_i}", name=f"psall{stat_i}")
            bia = sb.tile([128, CH], dt.float32, tag="pcbias", name="pcbias")
            nc.sync.dma_start(bia[:], convb_d.ap()[conv_i])
            ga = big.tile([128, NT, CH], dt.bfloat16, tag=f"pgelu{stat_i}", name=f"pgelu{stat_i}")
            a2a_t = a2a_ap.rearrange("r (t p) c -> r p t c", p=128)
            for t in range(NT):
                r0 = sb.tile([128, CH], dt.bfloat16, tag="pr0", name="pr0")
                nc.sync.dma_start(r0[:], a2a_t[0, :, t, :])
                r1 = sb.tile([128, CH], dt.bfloat16, tag="pr1", name="pr1")
                nc.sync.dma_start(r1[:], a2a_t[1, :, t, :])
                yt = sb.tile([128, CH], dt.float32, tag="pyt", name="pyt")
                nc.vector.tensor_tensor(yt[:], r0[:], r1[:], OP.add)
                nc.vector.tensor_tensor(yt[:], yt[:], giveS[:, t, :], OP.subtract)
                nc.vector.tensor_tensor(yt[:], yt[:], ymine[:, t, :], OP.add)
                nc.vector.tensor_tensor(yt[:], yt[:], bia[:], OP.add)
                gt = sb.tile([128, CH], dt.bfloat16, tag="pgt", name="pgt")
                gelu_act(gt[:], yt[:], sb)
                nc.vector.tensor_copy(ga[:, t, :], gt[:])
                nc.vector.reduce_sum(sall[:, t:t + 1], gt[:], axis=mybir.AxisListType.X)
                sq = sb.tile([128, CH], dt.float32, tag="psq", name="psq")
                nc.scalar.activation(sq[:], gt[:], AF.Square,
                                     accum_out=sall[:, NT + t:NT + t + 1])
            nc.sync.dma_start(stat_in[stat_i][:], sall[:])
            nc.gpsimd.collective_compute(
                "AllReduce", OP.add, replica_groups=RG,
                ins=[stat_in[stat_i].opt()], outs=[stat_out[stat_i].opt()])
            sfull = sb.tile([128, 2 * NT], dt.float32, tag="psfull", name="psfull")
            nc.sync.dma_start(sfull[:], stat_out[stat_i][:])
            mean = big.tile([128, NT], dt.float32, tag=f"pmean{stat_i}", name=f"pmean{stat_i}")
            nc.vector.tensor_scalar(mean[:], sfull[:, 0:NT], 1.0 / C, None, OP.mult)
            msq = sb.tile([128, NT], dt.float32, tag="pmsq", name="pmsq")
            nc.vector.tensor_tensor(msq[:], mean[:], mean[:], OP.mult)
            var = sb.tile([128, NT], dt.float32, tag="pvar", name="pvar")
            nc.vector.scalar_tensor_tensor(out=var[:], in0=sfull[:, NT:2 * NT],
                                           scalar=1.0 / C, in1=msq[:],
                                           op0=OP.mult, op1=OP.subtract)
            rs = big.tile([128, NT], dt.float32, tag=f"prs{stat_i}", name=f"prs{stat_i}")
            sdv = sb.tile([128, NT], dt.float32, tag="psdv", name="psdv")
            nc.scalar.activation(sdv[:], var[:], AF.Sqrt, bias=eps_col[:])
            nc.vector.reciprocal(rs[:], sdv[:])
            ln_nm = "ln2" if conv_i == 0 else "ln4"
            ln_i = 1 if conv_i == 0 else 3
            triv = ln_trivial[ln_nm]
            if not triv:
                sc = sb.tile([128, CH], dt.float32, tag="plnsc", name="plnsc")
                nc.sync.dma_start(sc[:], lnsc_d.ap()[ln_i, :, 0:CH])
                bi2 = sb.tile([128, CH], dt.float32, tag="plnbi", name="plnbi")
                nc.sync.dma_start(bi2[:], lnbi_d.ap()[ln_i, :, 0:CH])
            for t in range(NT):
                zt = sb.tile([128, CH], dt.float32, tag="pzt", name="pzt")
                nc.vector.tensor_scalar(zt[:], ga[:, t, :], mean[:, t:t + 1],
                                        rs[:, t:t + 1], OP.subtract, OP.mult)
                if not triv:
                    nc.vector.tensor_tensor(zt[:], zt[:], sc[:], OP.mult)
                    nc.vector.tensor_tensor(zt[:], zt[:], bi2[:], OP.add)
                nc.vector.tensor_copy(zout[:, t, :], zt[:])

        # ============================ Phase 1 + chain 1 ============================
        with tc.tile_pool(name="big1", bufs=1) as big, \
             tc.tile_pool(name="sb1", bufs=3) as sb, \
             tc.tile_pool(name="gb1", bufs=6) as gbp, \
             tc.tile_pool(name="ps1", bufs=4, space="PSUM") as psp:
            ring = [big.tile([128, NT, CH], dt.bfloat16, tag=f"ring{i}", name=f"ring{i}") for i in range(3)]
            triv1 = ln_trivial["ln1"]
            if not triv1:
                sc1 = big.tile([128, C], dt.float32, tag="l1sc", name="l1sc")
                nc.sync.dma_start(sc1[:], lnsc_d.ap()[0])
                bi1 = big.tile([128, C], dt.float32, tag="l1bi", name="l1bi")
                nc.sync.dma_start(bi1[:], lnbi_d.ap()[0])
            for t in range(NT):
                xt = sb.tile([128, C], dt.float32, tag="l1x", name="l1x")
                nc.sync.dma_start(xt[:], maps_t[:, t, :])
                st = sb.tile([128, 6], dt.float32, tag="l1st", name="l1st")
                nc.vector.bn_stats(st[:], xt[:])
                ag = sb.tile([128, 2], dt.float32, tag="l1ag", name="l1ag")
                nc.vector.bn_aggr(ag[:], st[:])
                rs = sb.tile([128, 1], dt.float32, tag="l1rs", name="l1rs")
                sd = sb.tile([128, 1], dt.float32, tag="l1sd", name="l1sd")
                nc.scalar.activation(sd[:], ag[:, 1:2], AF.Sqrt, bias=eps_col[:])
                nc.vector.reciprocal(rs[:], sd[:])
                zt = sb.tile([128, C], dt.float32, tag="l1z", name="l1z")
                nc.vector.tensor_scalar(zt[:], xt[:], ag[:, 0:1], rs[:],
                                        OP.subtract, OP.mult)
                if not triv1:
                    nc.vector.tensor_tensor(zt[:], zt[:], sc1[:], OP.mult)
                    nc.vector.tensor_tensor(zt[:], zt[:], bi1[:], OP.add)
                nc.vector.tensor_copy(ring[0][:, t, :], zt[:, 0:CH])
                nc.sync.dma_start(rows_ap[0].rearrange("(t p) c -> p t c", p=128)[:, t, :],
                                  ring[0][:, t, :])
                transpose_to_tT(ring[0][:, t, :], tT_ap[0], t, sb, psp)
            run_chain(ring, sb, gbp, psp)

        # ============================ conv1 + exchange + LN2 + token mix ============================
        if stop_stage >= 2:
            with tc.tile_pool(name="big3", bufs=1) as big, \
                 tc.tile_pool(name="sb3", bufs=3) as sb, \
                 tc.tile_pool(name="ps3", bufs=2, space="PSUM") as psp:
                ymine1 = big.tile([128, NT, CH], dt.bfloat16, tag="ymine1", name="ymine1")
                giveS1 = big.tile([128, NT, CH], dt.bfloat16, tag="giveS1", name="giveS1")
                conv_phase(w1h, give1[:], ymine1, giveS1, sb, psp)
                nc.gpsimd.collective_compute(
                    "AllGather", mybir.AluOpType.bypass, replica_groups=RG,
                    ins=[give1.opt()], outs=[a2a1.opt()])
                z1 = big.tile([128, NT, CH], dt.bfloat16, tag="z1", name="z1")
                postconv(a2a1[:], 0, 0, z1, ymine1, giveS1, big, sb)

                # token mixing
                ps1t = psp.tile([128, T], dt.float32, tag="tokps", name="tokps")
                for t in range(NT):
                    wt = sb.tile([128, T], dt.bfloat16, tag="tw1", name="tw1")
                    nc.sync.dma_start(wt[:], tokw1_d.ap().bitcast(dt.bfloat16)
                                      .rearrange("(t p) f -> p t f", p=128)[:, t, :])
                    nc.tensor.matmul(ps1t[:], z1[:, t, :], wt[:],
                                     start=(t == 0), stop=(t == NT - 1))
                b1t = sb.tile([128, T], dt.float32, tag="b1t", name="b1t")
                nc.sync.dma_start(b1t[:], b1bc_d.ap())
                h1 = sb.tile([128, T], dt.float32, tag="h1", name="h1")
                nc.vector.tensor_tensor(h1[:], ps1t[:], b1t[:], OP.add)
                h1g = big.tile([128, T], dt.bfloat16, tag="h1g", name="h1g")
                gelu_act(h1g[:], h1[:], sb)
                h1gT = big.tile([128, 4, 128], dt.bfloat16, tag="h1gT", name="h1gT")
                for q in range(4):
                    pt = psp.tile([128, 128], dt.bfloat16, tag="tokptr", name="tokptr")
                    nc.tensor.transpose(pt[:], h1g[:, q * 128:(q + 1) * 128], idn[:])
                    nc.scalar.activation(h1gT[:, q, :], pt[:], AF.Copy)
                tb2 = sb.tile([128, NT], dt.float32, tag="tb2", name="tb2")
                nc.sync.dma_start(tb2[:], tokb2_d.ap())
                for t in range(NT):
                    ps2 = psp.tile([128, CH], dt.float32, tag="tokps2", name="tokps2")
                    for q in range(4):
                        w2t = sb.tile([128, 128], dt.bfloat16, tag="tw2", name="tw2")
                        nc.sync.dma_start(w2t[:], tokw2_d.ap().bitcast(dt.bfloat16)
                                          [q * 128:(q + 1) * 128, t * 128:(t + 1) * 128])
                        nc.tensor.matmul(ps2[:], w2t[:], h1gT[:, q, :],
                                         start=(q == 0), stop=(q == 3))
                    mp = sb.tile([128, CH], dt.float32, tag="tokmp", name="tokmp")
                    nc.sync.dma_start(mp[:], maps_t[:, t, 0:CH])
                    x1t = sb.tile([128, CH], dt.float32, tag="x1t", name="x1t")
                    nc.vector.scalar_tensor_tensor(out=x1t[:], in0=ps2[:],
                                                   scalar=tb2[:, t:t + 1], in1=mp[:],
                                                   op0=OP.add, op1=OP.add)
                    nc.sync.dma_start(x1_t[:, t, :], x1t[:])

        # ============================ LN3 + chain 2 ============================
        if stop_stage >= 3:
            with tc.tile_pool(name="big6", bufs=1) as big, \
                 tc.tile_pool(name="sb6", bufs=3) as sb, \
                 tc.tile_pool(name="gb6", bufs=6) as gbp, \
                 tc.tile_pool(name="ps6", bufs=4, space="PSUM") as psp:
                sall = big.tile([128, 2 * NT], dt.float32, tag="l3sall", name="l3sall")
                for t in range(NT):
                    xt = sb.tile([128, CH], dt.float32, tag="l3xt", name="l3xt")
                    nc.sync.dma_start(xt[:], x1_t[:, t, :])
                    nc.vector.reduce_sum(sall[:, t:t + 1], xt[:], axis=mybir.AxisListType.X)
                    sq = sb.tile([128, CH], dt.float32, tag="l3sq", name="l3sq")
                    nc.scalar.activation(sq[:], xt[:], AF.Square,
                                         accum_out=sall[:, NT + t:NT + t + 1])
                nc.sync.dma_start(stat_in[1][:], sall[:])
                nc.gpsimd.collective_compute(
                    "AllReduce", OP.add, replica_groups=RG,
                    ins=[stat_in[1].opt()], outs=[stat_out[1].opt()])
                sfull = sb.tile([128, 2 * NT], dt.float32, tag="l3sfull", name="l3sfull")
                nc.sync.dma_start(sfull[:], stat_out[1][:])
                mean = big.tile([128, NT], dt.float32, tag="l3mean", name="l3mean")
                nc.vector.tensor_scalar(mean[:], sfull[:, 0:NT], 1.0 / C, None, OP.mult)
                msq = sb.tile([128, NT], dt.float32, tag="l3msq", name="l3msq")
                nc.vector.tensor_tensor(msq[:], mean[:], mean[:], OP.mult)
                var = sb.tile([128, NT], dt.float32, tag="l3var", name="l3var")
                nc.vector.scalar_tensor_tensor(out=var[:], in0=sfull[:, NT:2 * NT],
                                               scalar=1.0 / C, in1=msq[:],
                                               op0=OP.mult, op1=OP.subtract)
                rsx = big.tile([128, NT], dt.float32, tag="l3rs", name="l3rs")
                sdv3 = sb.tile([128, NT], dt.float32, tag="l3sdv", name="l3sdv")
                nc.scalar.activation(sdv3[:], var[:], AF.Sqrt, bias=eps_col[:])
                nc.vector.reciprocal(rsx[:], sdv3[:])
                triv3 = ln_trivial["ln3"]
                if not triv3:
                    sc3 = big.tile([128, CH], dt.float32, tag="l3sc", name="l3sc")
                    nc.sync.dma_start(sc3[:], lnsc_d.ap()[2, :, 0:CH])
                    bi3 = big.tile([128, CH], dt.float32, tag="l3bi", name="l3bi")
                    nc.sync.dma_start(bi3[:], lnbi_d.ap()[2, :, 0:CH])
                ring = [big.tile([128, NT, CH], dt.bfloat16, tag=f"ring2_{i}", name=f"ring2_{i}") for i in range(3)]
                for t in range(NT):
                    xt = sb.tile([128, CH], dt.float32, tag="l3xt2", name="l3xt2")
                    nc.sync.dma_start(xt[:], x1_t[:, t, :])
                    zt = sb.tile([128, CH], dt.float32, tag="l3zt", name="l3zt")
                    nc.vector.tensor_scalar(zt[:], xt[:], mean[:, t:t + 1],
                                            rsx[:, t:t + 1], OP.subtract, OP.mult)
                    if not triv3:
                        nc.vector.tensor_tensor(zt[:], zt[:], sc3[:], OP.mult)
                        nc.vector.tensor_tensor(zt[:], zt[:], bi3[:], OP.add)
                    nc.vector.tensor_copy(ring[0][:, t, :], zt[:])
                    nc.sync.dma_start(rows_ap[0].rearrange("(t p) c -> p t c", p=128)[:, t, :],
                                      ring[0][:, t, :])
                    transpose_to_tT(ring[0][:, t, :], tT_ap[0], t, sb, psp)
                run_chain(ring, sb, gbp, psp)

        # ============================ conv2 + exchange + LN4 + z4 + channel mix ============================
        if stop_stage >= 4:
            with tc.tile_pool(name="big8", bufs=1) as big, \
                 tc.tile_pool(name="sb8", bufs=3) as sb, \
                 tc.tile_pool(name="ps8", bufs=1, space="PSUM") as psp:
                ymine2 = big.tile([128, NT, CH], dt.bfloat16, tag="ymine2", name="ymine2")
                giveS2 = big.tile([128, NT, CH], dt.bfloat16, tag="giveS2", name="giveS2")
                conv_phase(w2h, give2[:], ymine2, giveS2, sb, psp)
                nc.gpsimd.collective_compute(
                    "AllGather", mybir.AluOpType.bypass, replica_groups=RG,
                    ins=[give2.opt()], outs=[a2a2.opt()])
                z4 = big.tile([128, NT, CH], dt.bfloat16, tag="z4", name="z4")
                postconv(a2a2[:], 1, 2, z4, ymine2, giveS2, big, sb)
                z4g_t = z4give[:].rearrange("(t p) c -> p t c", p=128)
                for t in range(NT):
                    nc.sync.dma_start(z4g_t[:, t, :], z4[:, t, :])
                nc.gpsimd.collective_compute(
                    "AllGather", mybir.AluOpType.bypass, replica_groups=RG,
                    ins=[z4give.opt()], outs=[a2az4.opt()])

                chw1s = big.tile([128, 2, CHID], dt.bfloat16, tag="chw1s", name="chw1s")
                nc.sync.dma_start(chw1s[:], chw1_d.ap().bitcast(dt.bfloat16)
                                  .rearrange("(u p) f -> p u f", p=128))
                chb1s = big.tile([128, 8], dt.float32, tag="chb1s", name="chb1s")
                nc.sync.dma_start(chb1s[:], chb1_d.ap())
                chw2s = big.tile([128, 8, CH], dt.bfloat16, tag="chw2s", name="chw2s")
                nc.sync.dma_start(chw2s[:], chw2h_d.ap().bitcast(dt.bfloat16)
                                  .rearrange("(u p) f -> p u f", p=128))
                chb2s = big.tile([128, CH], dt.float32, tag="chb2s", name="chb2s")
                nc.sync.dma_start(chb2s[:], chb2bc_d.ap())
                a2az4_t = a2az4[:].rearrange("r (t p) c -> r p t c", p=128)
                outp_t = outp.ap().rearrange("(t p) c -> p t c", p=128)

                NCH = 4
                for cstart in range(0, NT, NCH):
                    z4T = big.tile([128, 2, NCH * 128], dt.bfloat16, tag="z4T", name="z4T")
                    for tl in range(NCH):
                        t = cstart + tl
                        for p in range(2):
                            prt = sb.tile([128, CH], dt.bfloat16, tag="chprt", name="chprt")
                            nc.sync.dma_start(prt[:], a2az4_t[p, :, t, :])
                            pt = psp.tile([128, 128], dt.bfloat16, tag="chptr", name="chptr")
                            nc.tensor.transpose(pt[:], prt[:], idn[:])
                            nc.scalar.activation(z4T[:, p, tl * 128:(tl + 1) * 128],
                                                 pt[:], AF.Copy)
                    h1cm = big.tile([128, 8, NCH * 128], dt.bfloat16, tag="h1cm", name="h1cm")
                    for m8 in range(8):
                        psc = psp.tile([128, NCH * 128], dt.float32, tag="chps", name="chps")
                        for u in range(2):
                            nc.tensor.matmul(psc[:], chw1s[:, u, m8 * 128:(m8 + 1) * 128],
                                             z4T[:, u, :], start=(u == 0), stop=(u == 1))
                        gelu_act(h1cm[:, m8, :], psc[:], sb, bias=chb1s[:, m8:m8 + 1])
                    for tl in range(NCH):
                        t = cstart + tl
                        ps2 = psp.tile([128, CH], dt.float32, tag="chps2", name="chps2")
                        for m8 in range(8):
                            nc.tensor.matmul(ps2[:], h1cm[:, m8, tl * 128:(tl + 1) * 128],
                                             chw2s[:, m8, :], start=(m8 == 0), stop=(m8 == 7))
                        x1t = sb.tile([128, CH], dt.float32, tag="chx1", name="chx1")
                        nc.sync.dma_start(x1t[:], x1_t[:, t, :])
                        ot = sb.tile([128, CH], dt.float32, tag="chot", name="chot")
                        nc.vector.tensor_tensor(ot[:], ps2[:], x1t[:], OP.add)
                        nc.vector.tensor_tensor(ot[:], ot[:], chb2s[:], OP.add)
                        nc.sync.dma_start(outp_t[:, t, :], ot[:])

        es.close()

    nc.compile()
    return nc


def kernel(**inputs):
    from concourse import bass_utils
    in_maps, ln_trivial = host_prep(inputs)

    key = ("nc8", tuple(sorted(ln_trivial.items())))
    if key not in _cache:
        _cache[key] = build_nc(NCORES, ln_trivial)
    nc = _cache[key]

    res = bass_utils.run_bass_kernel_spmd(nc, in_maps, core_ids=list(range(NCORES)))

    out = np.zeros((B, N, C), np.float32)
    for cid in range(NCORES):
        b, h = cid // 2, cid % 2
        out[b, :, h * CH:(h + 1) * CH] = res.results[cid]["outp"]
    return out

